# revision 88
# baseline (speedup 1.0000x reference)
"""DSS (Diagonal State Space) layer as a Bass/Tile kernel for 8 Trainium2 NeuronCores.

Algorithm (per core, channels H sharded 8 x 128):
  1. Build the DSS-exp kernel k[l,h] = Re(sum_n W[h,n] z[h,n]^l), z = exp(dt_h * Lambda_n),
     on-device via a two-level power factorization l = 32a + b:
       GW[h,n,b] = W * z^b (b<32),  Z32[h,n,a] = z^(32a) (a<16), both by complex doubling,
     then a per-channel PE matmul contracts the 64 modes. The whole prologue runs in a
     split-H layout (partitions = (h//64, n)) so every elementwise op uses all 128
     partitions; chain work is balanced DVE/Pool so both finish together (~17us each).
  2. K_f = rfft_1024(k) via PE matmuls against host-precomputed DFT tiles.
  3. Overlap-save FFT convolution with HALF-WINDOW SHARING: each 512-sample half-window's
     1024-point DFT contribution P_m is computed once (4 chunk matmuls x 2 parts x 4 freq
     tiles) and reused by the two blocks that contain it: U_b = P_{b-1} + (-1)^f P_b.
     Frequencies are packed even/odd (tiles 0,1 = even f, tiles 2,3 = odd f) so the
     (-1)^f factor is a constant +1/-1 per tile and the combine is one fp16 add/subtract.
  4. The skip connection y += u * D is folded into the frequency-domain filter
     (K'_f = K_f + D), so it costs nothing in the main loop.

Precision plan (rel err ~3.7e-3 vs the 2e-2 gate):
  - The FORWARD DFT runs in fp8-e4m3 DoubleRow matmuls (0.5 cycles/row, 2x fp16
    rate) with a residual split: u = u8 + r8 (both e4m3, one merged transfer),
    CF = CFhi + CFlo. P = (CFhi|CFlo)^T(u8,u8) + (CFhi_c0|CFhi_c1)^T(r8_c0,r8_c1)
    per chunk pair -- 6 DoubleRow matmuls per 256-col psum region, 25% less PE
    time than fp16, with ~3e-3 spectral error instead of fp8's raw ~3e-2.
  - Everything else runs fp16 (same PE/DVE rates as bf16, 8x less quantization
    error). A global 1/4 scale is folded into CF and compensated x16 in the
    inverse AI stationaries, keeping products < fp16 max. The mode-sum operands
    (GW, Z32 planes) are fp16 too: fp16 matmuls take 1 cycle/row where float32r
    small-free matmuls took 4 (the power/trig chains stay fp32 -- only the final
    plane writes quantize).

Pointwise runs on ft-PAIR tensors (pm/up/t/yf are [128, 2, ...]): combines,
products (4D-broadcast K-pair multipliers) and sums are 1024-2048-element fused
ops, which cuts DVE/Pool op count ~2x and element work ~8%.

Host-side layouts are chunk-major and fully contiguous, so every DMA moves
>=512B lines at full bus rate. Because HWDGE issues all hardware-DGE DMAs
through ONE serial ~630ns/slot resource, transfers are merged: params ride one
[1,384] row, u8+r8 load as one tile per chunk pair, CF8 as per-ft mega-tiles
(whose hl axis strides across chunks, so the cross-chunk hi-pair stationary is
a free view), Wre/Wim as one tile, AI as per-ft pairs.

Schedule: warm-up matmuls keep PE busy from ~1.1us so the p-state ramp is paid
once. The K pipeline is front-loaded: mode-sum splits half 4 (the Z32 chain
tail ends ~37us), kp evacuations ride ACT, K copies ride DVE (each would stall
PE behind the other engine's queue otherwise), kdft right after. The block loop
is software-pipelined: A(blk) = combine+pointwise on DVE/Pool runs a block
ahead of B(blk) = inverse matmuls + ACT evacuation + y DMA; halves 5-7 slot
between the first inverse blocks as PE-side cushions while the A-stream primes.
The last output tile runs as two uneven accumulation groups in separate psum
banks so the final DMA's fixed issue+semaphore latency dominates the tail.
"""

import sys

for _p in ("/opt/trn_rl_repo", "/opt/trn_rl_repo/concourse"):
    if _p not in sys.path:
        sys.path.insert(0, _p)

import numpy as np
import ml_dtypes
from contextlib import ExitStack

import concourse.bacc as bacc
import concourse.tile as tile
import concourse.mybir as mybir

dt = mybir.dt
f32 = np.float32
f16 = np.float16
f8 = ml_dtypes.float8_e4m3
DR = mybir.MatmulPerfMode.DoubleRow

B, L, H, N = 4, 4096, 1024, 64
LK = 512
F = 1024          # FFT length (overlap-save)
HOP = 512         # block hop
NCORES = 8
HS = H // NCORES  # 128 channels per core
NBLK = L // HOP   # 8
NFT = 4           # packed frequency tiles (even: 0,1; odd: 2,3; Nyquist folded)
NJ = 4            # contraction chunks per half-window
NLT = HOP // 128  # 4 output l-tiles per block
NCH = L // 128    # 32 u chunks per core
NHALF = L // HOP  # 8 half-windows
HG = 64           # channels per split-H partition group (prologue layout)
SCALE = 4.0       # CF * 1/SCALE, AI * SCALE^2, D-injection * 1/SCALE
N_WARM0 = 43      # PE warm-up matmuls before the first real matmul
N_WARMJ = 0       # gap-filler matmuls between half-0 j-groups


# ---------------------------------------------------------------- host constants
def _freq_perm():
    perm = np.zeros((NFT, 128), dtype=np.int64)
    r = np.arange(128)
    perm[0] = 2 * r
    perm[1] = 2 * (128 + r)
    perm[2] = 2 * r + 1
    perm[3] = 2 * (128 + r) + 1
    return perm


def build_constants():
    perm = _freq_perm()
    l = np.arange(HOP, dtype=np.float64)
    CF = np.zeros((NJ, 2, NFT, 128, 128))
    for j in range(NJ):
        lj = l[128 * j:128 * j + 128][:, None]
        for ft in range(NFT):
            f = perm[ft][None, :].astype(np.float64)
            ang = 2 * np.pi * lj * f / F
            CF[j, 0, ft] = np.cos(ang)
            CF[j, 1, ft] = -np.sin(ang)
        # Nyquist row packed into the f=0 slot of the sin tile (even group, sign +1)
        CF[j, 1, 0][:, 0] = (-1.0) ** l[128 * j:128 * j + 128]
    lc = HOP + np.arange(HOP, dtype=np.float64)[None, :]   # valid circular outputs
    AI = np.zeros((2, NFT, NLT, 128, 128))
    for ft in range(NFT):
        fr = perm[ft][:, None].astype(np.float64)
        cf_ = np.where(fr == 0, 1.0, 2.0)
        Ar = cf_ * np.cos(2 * np.pi * fr * lc / F) / F
        Ai = -(2.0 / F) * np.sin(2 * np.pi * fr * lc / F)
        if ft == 0:
            Ai[0, :] = ((-1.0) ** lc[0]) / F                # Nyquist inverse row
        for lt in range(NLT):
            AI[0, ft, lt] = Ar[:, 128 * lt:128 * lt + 128]
            AI[1, ft, lt] = Ai[:, 128 * lt:128 * lt + 128]
    CF *= 1.0 / SCALE
    AI *= SCALE * SCALE
    # SBUF-tile-major merged layouts (one contiguous DMA each):
    # CF [j, l(128), t, ft, f] (fp16, kdft only) ; AI [ft, f(128), t, lt, l]
    CFd = np.ascontiguousarray(CF.transpose(0, 3, 1, 2, 4)).astype(f16)
    AId = np.ascontiguousarray(AI.transpose(1, 3, 0, 2, 4)).astype(f16)
    # fp8 hi/lo residual split of CF for the DoubleRow forward:
    # CF8 [ft, l(128), t, j, hl(2), f]
    CFhi = CF.astype(f8).astype(np.float64)
    CFlo = (CF - CFhi).astype(f8)
    CF8 = np.stack([CFhi.astype(f8), CFlo], axis=4)   # [j, t, ft, 128, hl, f]
    CF8d = np.ascontiguousarray(CF8.transpose(2, 3, 1, 0, 4, 5))
    return CFd, AId, CF8d


# Horner coefficient lists (highest degree first)
def _fact(k):
    r = 1.0
    for i in range(2, k + 1):
        r *= i
    return r


EXP10 = [1.0 / _fact(k) for k in range(7, -1, -1)]          # e^x, |x| <~ 0.9
EXP9 = [1.0 / _fact(k) for k in range(6, -1, -1)]            # e^x, |x| <~ 0.4
SIN9 = [1.0 / _fact(9), -1.0 / _fact(7), 1.0 / _fact(5), -1.0 / _fact(3), 1.0]   # odd, in u = x^2
COSC = [-1.0 / _fact(10), 1.0 / _fact(8), -1.0 / _fact(6), 1.0 / _fact(4),
        -1.0 / _fact(2), 1.0]     # cos(x) - 1 in u = x^2, b1 first


class _Prog:
    def __init__(self):
        self.nc = None
        self.built = False


_prog = _Prog()


def _emit_kernel(nc, tc, ctx, aps):
    V = nc.vector
    A = nc.scalar
    T = nc.tensor
    GP = nc.gpsimd
    u_ap = aps["u"]; y_ap = aps["y"]
    cf_ap = aps["CF"]; ai_ap = aps["AI"]
    TT = V.tensor_tensor
    GT = GP.tensor_tensor
    op = mybir.AluOpType

    # ---------------- pools
    p_cf = ctx.enter_context(tc.tile_pool(name="cf", bufs=1))
    p_ai = ctx.enter_context(tc.tile_pool(name="ai", bufs=1))
    p_uch = ctx.enter_context(tc.tile_pool(name="uch", bufs=6))
    p_pm = ctx.enter_context(tc.tile_pool(name="pm", bufs=14))
    p_u = ctx.enter_context(tc.tile_pool(name="u", bufs=6))
    p_yf = ctx.enter_context(tc.tile_pool(name="yf", bufs=10))
    p_krep = ctx.enter_context(tc.tile_pool(name="krep", bufs=1))
    p_tmp = ctx.enter_context(tc.tile_pool(name="tmp", bufs=6))
    p_yout = ctx.enter_context(tc.tile_pool(name="yout", bufs=3))
    p_kc = ctx.enter_context(tc.tile_pool(name="kc", bufs=1))
    p_gw = ctx.enter_context(tc.tile_pool(name="gw", bufs=1))
    p_z32 = ctx.enter_context(tc.tile_pool(name="z32", bufs=1))
    p_zp = ctx.enter_context(tc.tile_pool(name="zp", bufs=9))
    p_small = ctx.enter_context(tc.tile_pool(name="small", bufs=1))
    p_gwtmp = ctx.enter_context(tc.tile_pool(name="gwtmp", bufs=1))
    p_ks = ctx.enter_context(tc.tile_pool(name="ks", bufs=1))
    p_psf = ctx.enter_context(tc.tile_pool(name="psf", bufs=2, space="PSUM"))
    p_psi = ctx.enter_context(tc.tile_pool(name="psi", bufs=2, space="PSUM"))
    p_psk = ctx.enter_context(tc.tile_pool(name="psk", bufs=2, space="PSUM"))

    # ---------------- PE warm-up: keep the tensor engine "continuously busy"
    # from its first possible slot so the p-state ramp completes once.
    wmS = p_small.tile([1, 64], dt.float16, tag="wmS")
    GP.memset(wmS[:], 1.0)
    wmM = p_small.tile([1, 64], dt.float16, tag="wmM")
    GP.memset(wmM[:], 1.0)
    ps_w = p_psk.tile([64, 64], dt.float32, tag="psk", name="warm")

    def warm(n):
        for _ in range(n):
            T.matmul(ps_w[:], wmS[:], wmM[:], start=True, stop=True)

    warm(N_WARM0)

    # ---------------- startup DMAs: HWDGE serializes ALL hwdge queues at one
    # ~630ns slot per DMA, so the startup set is merged into few transfers,
    # ordered by first PE need. The Pool SWDGE queue is left empty so the
    # Lambda chain gets the Pool engine immediately.
    par = p_small.tile([1, 384], dt.float32, tag="par")
    nc.sync.dma_start(par[:], aps["par"][:])
    logdt = par[:, 0:HS]
    Lre = par[:, HS:HS + N]
    Lim_v = par[:, HS + N:HS + 2 * N]
    dtile = par[:, HS + 2 * N:HS + 2 * N + HS]

    cf_big = {}

    def load_cf(j, eng):
        tl = p_cf.tile([128, 2, 4, 128], dt.float16, tag=f"cf{j}", name=f"cfb{j}")
        eng.dma_start(tl[:], cf_ap[j])
        cf_big[j] = tl

    def cf_tile(j, t_, ft):
        return cf_big[j][:, t_, ft, :]

    cf8_big = {}

    def load_cf8(ft, eng):
        tl = p_cf.tile([128, 2, 4, 2, 128], dt.float8e4, tag=f"cf8{ft}", name=f"cf8b{ft}")
        eng.dma_start(tl[:], aps["CF8"][ft])
        cf8_big[ft] = tl

    def cf8_hl(ft, t_, j):
        # DoubleRow stationary (CFhi, CFlo) for chunk j
        return cf8_big[ft][:, t_, j, :, :]

    def cf8_hh(ft, t_, jp):
        # DoubleRow stationary (CFhi_c2p, CFhi_c2p+1): the hl axis of the
        # mega-tile strides across j, so the cross-chunk hi pair is a free view
        return cf8_big[ft][:, t_, 2 * jp:2 * jp + 2, 0, :]

    chunks = {}

    def get_chunk(c, eng=None):
        # one merged [u8 | r8] transfer per chunk pair
        p = c // 2
        if p not in chunks:
            t8 = p_uch.tile([128, 2, 2, 4, 128], dt.float8e4, tag="uch", name=f"u8p{p}")
            if eng is None:
                # SP only: a chunk DMA on ACT's SEQ delays the psf evacuations
                # that pace the forward
                eng = nc.sync
            eng.dma_start(t8[:], u_ap[p])
            chunks[p] = t8
        return chunks[p]

    load_cf8(0, nc.sync)
    get_chunk(0, nc.sync)       # chunk pair 0
    load_cf8(1, nc.scalar)
    get_chunk(2, nc.sync)       # pair 1 right behind pair 0 on SP
    load_cf8(2, GP)             # cf8 pair 2 via Pool SWDGE
    load_cf8(3, nc.sync)
    get_chunk(4, nc.sync)       # prefetch half 1's pairs
    get_chunk(6, nc.scalar)

    # ---------------- forward: one half-window spectrum P_m per 512 samples.
    # P is stored in FREQ-TILE-PAIR tiles [128, 2, 1024] (pairs (ft0,ft1) and
    # (ft2,ft3)): the combine/pointwise then run as wide fused ops and the
    # (-1)^f combine sign is constant per pair.
    halves = {}

    def emit_half(m, fts=range(NFT), pad=0):
        # cos/sin accumulation groups share one 2-bank psum tile per ft, so the
        # evacuation is a single wide ACT copy
        pair = halves.setdefault(m, {})
        for ft in fts:
            pp = p_psf.tile([128, 1024], dt.float32, tag="psf", name=f"pp{m}_{ft}")
            # the cos half [0:512] and sin half [512:1024] are different psum
            # banks, so their 256-col accumulation groups can run interleaved
            # j-major -- the matmul stream then consumes chunks in DMA-arrival
            # order instead of needing the whole half-window up front
            for h in range(2):
                def reg(t_):
                    return pp[:, 512 * t_ + 256 * h:512 * t_ + 256 * h + 256]

                for j in range(NJ):
                    u8t = get_chunk(4 * m + j)
                    u8c = u8t[:, 0, j % 2, :, :].rearrange("p b h -> p (b h)")
                    mov = (u8c[:, 256 * h:256 * h + 256].unsqueeze(1)
                           .broadcast_to([128, 2, 256]))
                    for t_ in range(2):
                        T.matmul(reg(t_), cf8_hl(ft, t_, j), mov,
                                 start=(j == 0), stop=False, perf_mode=DR)
                for jp in range(2):
                    r8t = get_chunk(4 * m + 2 * jp)
                    r8c = r8t[:, 1, :, :, :].rearrange("p c b h -> p c (b h)")
                    for t_ in range(2):
                        T.matmul(reg(t_), cf8_hh(ft, t_, jp),
                                 r8c[:, :, 256 * h:256 * h + 256],
                                 start=False, stop=(jp == 1), perf_mode=DR)
                if pad:
                    warm(pad)
            if ft // 2 not in pair:
                pair[ft // 2] = p_pm.tile([128, 2, 1024], dt.float16, tag="pm",
                                          name=f"pm{m}_{ft // 2}")
            A.copy(pair[ft // 2][:, ft % 2, :], pp[:])

    # half 0 ft0 alone first: its j-groups arrive at the startup DMA rate, so a
    # narrow consumer (2 matmuls per j + warm padding) avoids long PE stalls
    emit_half(0, fts=range(0, 1), pad=N_WARMJ)

    def horner_exp(dst, x, coefs, eng=None, tag="horner"):
        # dst = e^x = 1 + x*Q(x). On DVE each Horner step p <- (p + b)*x is one
        # fused scalar_tensor_tensor op; the Pool engine has no STT on hardware,
        # so it uses the classic two-op form there.
        tsm = eng.tensor_scalar_mul if eng is not None else V.tensor_scalar_mul
        tsa = eng.tensor_scalar_add if eng is not None else V.tensor_scalar_add
        p = dst
        tsm(p, x, float(coefs[0]))
        for c in coefs[1:-1]:
            if eng is None:
                V.scalar_tensor_tensor(p, p, float(c), x, op.add, op.mult)
            else:
                tsa(p, p, float(c))
                eng.tensor_tensor(p, p, x, op.mult)
        tsa(p, p, 1.0)

    # dt = exp(logdt) = (exp(logdt/8))^8   (DVE chain)
    x8 = p_small.tile([1, HS], dt.float32, tag="x8")
    V.tensor_scalar_mul(x8[:], logdt, 0.125)
    e8 = p_small.tile([1, HS], dt.float32, tag="e8")
    horner_exp(e8[:], x8[:], EXP10, tag="h_dt")
    dtv = p_small.tile([1, HS], dt.float32, tag="dtv")
    t_a = p_small.tile([1, HS], dt.float32, tag="sq1")
    TT(t_a[:], e8[:], e8[:], op.mult)
    t_b = p_small.tile([1, HS], dt.float32, tag="sq2")
    TT(t_b[:], t_a[:], t_a[:], op.mult)
    TT(dtv[:], t_b[:], t_b[:], op.mult)

    # -exp(Lre) = -(exp(Lre/8))^8   (Pool chain, concurrent with the dt chain)
    xl = p_small.tile([1, N], dt.float32, tag="xl")
    GP.tensor_scalar_mul(xl[:], Lre, 0.125)
    el8 = p_small.tile([1, N], dt.float32, tag="el8")
    horner_exp(el8[:], xl[:], EXP10, eng=GP, tag="h_lre")
    t_c = p_small.tile([1, N], dt.float32, tag="sq3")
    GT(t_c[:], el8[:], el8[:], op.mult)
    t_d = p_small.tile([1, N], dt.float32, tag="sq4")
    GT(t_d[:], t_c[:], t_c[:], op.mult)
    negel = p_small.tile([1, N], dt.float32, tag="negel")
    t_e = p_small.tile([1, N], dt.float32, tag="sq5")
    GT(t_e[:], t_d[:], t_d[:], op.mult)
    GP.tensor_scalar_mul(negel[:], t_e[:], -1.0)

    # half-0 ft1 first: the fp8 forward reaches here ~2us sooner than negel
    # lands, so the outer products slot in after it
    emit_half(0, fts=range(1, 2))

    # outer products in split-H layout, as early as negel allows so the trig
    # and power chains start promptly: partitions = (hg, n), hg = h//64
    ps_a = p_psk.tile([128, 512], dt.float32, tag="psk")
    ps_b = p_psk.tile([128, 512], dt.float32, tag="psk")
    for hg in range(2):
        T.matmul(ps_a[64 * hg:64 * hg + 64, 0:HG], negel[:],
                 dtv[:, HG * hg:HG * hg + HG], start=True, stop=True)
        T.matmul(ps_b[64 * hg:64 * hg + 64, 0:HG], Lim_v,
                 dtv[:, HG * hg:HG * hg + HG], start=True, stop=True)

    # remaining forward halves + inverse stationaries queue behind the outer products
    emit_half(0, fts=range(2, NFT))
    # W rides here (needed for the GW seed planes at ~12us): any earlier and it
    # delays half 1's chunk pairs in the serial HWDGE queue
    wt = p_small.tile([128, 2, HG], dt.float32, tag="wt")
    A.dma_start(wt[:], aps["W2"][:])
    wret = wt[:, 0, :]
    wimt = wt[:, 1, :]
    for m in range(1, 4):
        emit_half(m)
    # fp16 CF tiles are only contracted by the kdft (~45us in): load them
    # behind the half 1-3 chunk traffic
    load_cf(0, nc.sync)
    load_cf(1, nc.scalar)
    load_cf(2, nc.sync)
    load_cf(3, nc.scalar)
    ai_big = {}
    for ft in range(NFT):
        tl = p_ai.tile([128, 2, 4, 128], dt.float16, tag=f"ai{ft}", name=f"aib{ft}")
        nc.sync.dma_start(tl[:], ai_ap[ft])
        ai_big[ft] = tl
    # prefetch halves 4/5's chunk pairs now -- emitted any later, their HWDGE
    # slots fall behind the kcb shuffle and AI tiles and PE stalls mid-half-5
    for c in (16, 18, 20, 22):
        get_chunk(c)

    def ai_tile(t_, ft, lt):
        return ai_big[ft][:, t_, lt, :]

    # half-angle pieces on [128, HG]; everything on DVE (fused STT Horner) --
    # this short serial chain gates both power chains, and Pool's 0.42-efficiency
    # two-op Horner would stretch it 3x
    ah = p_small.tile([128, HG], dt.float32, tag="ah")
    V.tensor_scalar_mul(ah[:], ps_a[0:128, 0:HG], 0.5)   # DVE: Pool cannot read PSUM
    bh = p_small.tile([128, HG], dt.float32, tag="bh")
    V.tensor_scalar_mul(bh[:], ps_b[0:128, 0:HG], 0.5)
    ea = p_small.tile([128, HG], dt.float32, tag="ea")
    horner_exp(ea[:], ah[:], EXP9, tag="h_ea")
    # sin(bh), cos(bh) via u = bh^2
    ub = p_small.tile([128, HG], dt.float32, tag="ub")
    TT(ub[:], bh[:], bh[:], op.mult)
    sp = p_small.tile([128, HG], dt.float32, tag="sp")
    V.tensor_scalar_mul(sp[:], ub[:], float(SIN9[0]))
    for c in SIN9[1:-1]:
        V.scalar_tensor_tensor(sp[:], sp[:], float(c), ub[:], op.add, op.mult)
    V.tensor_scalar_add(sp[:], sp[:], 1.0)
    sb = p_small.tile([128, HG], dt.float32, tag="sb")
    TT(sb[:], sp[:], bh[:], op.mult)          # sin(b/2)
    cb = p_small.tile([128, HG], dt.float32, tag="cb")
    V.tensor_scalar_mul(cb[:], ub[:], float(COSC[0]))
    for c in COSC[1:-1]:
        V.scalar_tensor_tensor(cb[:], cb[:], float(c), ub[:], op.add, op.mult)
    V.tensor_scalar_add(cb[:], cb[:], 1.0)

    wre = p_small.tile([128, HG], dt.float32, tag="wre")
    TT(wre[:], ea[:], cb[:], op.mult)
    wim = p_small.tile([128, HG], dt.float32, tag="wim")
    TT(wim[:], ea[:], sb[:], op.mult)

    # GW/Z32 plane tiles are fp16 (the mode-sum matmul runs at 1 cycle/row);
    # the z power chains below stay fp32 so squaring error does not compound.
    GWre_t = p_gw.tile([128, HG, 32], dt.float16, tag="gwre")
    GWim_t = p_gw.tile([128, HG, 32], dt.float16, tag="gwim")   # stores -Im
    GWre = GWre_t[:]
    GWim = GWim_t[:]
    V.tensor_copy(GWre[:, :, 0], wret)
    V.tensor_scalar_mul(GWim[:, :, 0], wimt, -1.0)

    # complex squaring on separate re/im planes (all base-partition 0, lane-aligned)
    def csq_parts(dre, dim_, sre, sim, eng=None):
        # re' = (re+im)(re-im), im' = (2 re) im -- 4-5 ops (STT fused on DVE;
        # the Pool engine has no STT on hardware)
        tt = eng.tensor_tensor if eng is not None else TT
        sfx = "p" if eng is not None else ""
        t1 = p_small.tile([128, HG], dt.float32, tag=f"csq1{sfx}", bufs=2)
        tt(t1[:], sre, sim, op.add)
        t2 = p_small.tile([128, HG], dt.float32, tag=f"csq2{sfx}", bufs=2)
        tt(t2[:], sre, sim, op.subtract)
        if eng is None:
            V.scalar_tensor_tensor(dim_, sre, 2.0, sim, op.mult, op.mult)
        else:
            t3 = p_small.tile([128, HG], dt.float32, tag=f"csq3{sfx}", bufs=2)
            tt(t3[:], sre, sim, op.mult)
            eng.tensor_scalar_mul(dim_, t3[:], 2.0)
        tt(dre, t1[:], t2[:], op.mult)

    def new_zpair(nm):
        zr = p_zp.tile([128, HG], dt.float32, tag="zp", name=f"{nm}r")
        zi = p_zp.tile([128, HG], dt.float32, tag="zp", name=f"{nm}i")
        return zr, zi

    def cdouble_seg(pre, pim, zr, zi, s0, d0, w, conj_stored, prim=None, sec=None):
        # planes [.., d0:d0+w] = planes[.., s0:s0+w] * (zr + i zi);
        # when conj_stored, the im plane holds the negated imaginary part.
        ptt = prim.tensor_tensor if prim is not None else TT
        stt = sec.tensor_tensor if sec is not None else GT
        tg = "zt2" if prim is not None else "gt2"
        zre = zr[:].unsqueeze(2).broadcast_to([128, HG, w])
        zim = zi[:].unsqueeze(2).broadcast_to([128, HG, w])
        t2 = p_gwtmp.tile([128, HG, 8], dt.float32, tag=tg, bufs=2)
        t4 = p_gwtmp.tile([128, HG, 8], dt.float32, tag=tg, bufs=2)
        ptt(pre[:, :, d0:d0 + w], pre[:, :, s0:s0 + w], zre, op.mult)
        stt(t2[:, :, 0:w], pim[:, :, s0:s0 + w], zim, op.mult)
        ptt(pim[:, :, d0:d0 + w], pim[:, :, s0:s0 + w], zre, op.mult)
        stt(t4[:, :, 0:w], pre[:, :, s0:s0 + w], zim, op.mult)
        ptt(pre[:, :, d0:d0 + w], pre[:, :, d0:d0 + w], t2[:, :, 0:w],
            op.add if conj_stored else op.subtract)
        ptt(pim[:, :, d0:d0 + w], pim[:, :, d0:d0 + w], t4[:, :, 0:w],
            op.subtract if conj_stored else op.add)

    def cdouble(pre, pim, zr, zi, w, conj_stored, prim=None, sec=None):
        cdouble_seg(pre, pim, zr, zi, 0, w, w, conj_stored, prim=prim, sec=sec)

    # ---------------- Z32 planes [(hg n), HG, 16] natural complex z^(32a)
    Zre_t = p_z32.tile([128, HG, 16], dt.float16, tag="z32re")
    Zim_t = p_z32.tile([128, HG, 16], dt.float16, tag="z32im")
    Zre = Zre_t[:]
    Zim = Zim_t[:]
    # a=0 plane is the complex constant 1+0i
    V.tensor_scalar(Zre[:, :, 0], wre[:], 0.0, 1.0, op.mult, op.add)
    V.tensor_scalar(Zim[:, :, 0], wre[:], 0.0, 0.0, op.mult, op.add)

    # interleaved power chain + doubling. Engine split tuned so DVE and Pool
    # chain streams both land ~17us: GW doubling DVE-primary/Pool-secondary,
    # Z32 levels w=1,2 Pool-primary (cheap there), w=4,8 DVE-primary (Pool's
    # 0.42-efficiency makes wide levels 3.4x dearer), za squarings on Pool.
    zp = []
    z0 = new_zpair("z0")
    csq_parts(z0[0][:], z0[1][:], wre[:], wim[:])
    zp.append(z0)
    cdouble(GWre, GWim, zp[0][0], zp[0][1], 1, conj_stored=True)
    for j in range(1, 5):                     # z^2, z^4, z^8, z^16
        zj = new_zpair(f"z{1 << j}")
        csq_parts(zj[0][:], zj[1][:], zp[-1][0][:], zp[-1][1][:])
        zp.append(zj)
        if j < 4:
            cdouble(GWre, GWim, zp[j][0], zp[j][1], 1 << j, conj_stored=True)
    # the za squarings are a SERIAL latency chain (za1 <- za0 <- ...), so they
    # run on DVE (508 vs Pool's 1110 ns per level); the wide Z32 doubling
    # levels are Pool-primary so Pool and DVE finish their chain streams
    # together (~30us)
    za = []
    z32t = new_zpair("z32")
    csq_parts(z32t[0][:], z32t[1][:], zp[4][0][:], zp[4][1][:])
    za.append(z32t)                           # z^32
    cdouble_seg(GWre, GWim, zp[4][0], zp[4][1], 0, 16, 8, conj_stored=True)
    cdouble_seg(GWre, GWim, zp[4][0], zp[4][1], 8, 24, 8, conj_stored=True)
    cdouble(Zre, Zim, za[0][0], za[0][1], 1, conj_stored=False, prim=GP, sec=V)
    for j in range(1, 4):                     # z^64, z^128, z^256
        zj = new_zpair(f"za{j}")
        csq_parts(zj[0][:], zj[1][:], za[-1][0][:], za[-1][1][:])
        za.append(zj)
        if j < 3:
            cdouble(Zre, Zim, za[j][0], za[j][1], 1 << j, conj_stored=False, prim=GP, sec=V)
        else:
            # the widest (and last) Z32 level runs DVE-primary: it is the
            # chain's tail and Pool would stretch it 3.6x
            cdouble(Zre, Zim, za[j][0], za[j][1], 1 << j, conj_stored=False, prim=V, sec=GP)

    # half 4's first freq pair keeps PE fed while the Z32 chain tail finishes;
    # the mode-sum slots in after it and the kcb shuffle DMAs then overlap
    # half 4's remaining matmuls
    emit_half(4, fts=range(0, 2))

    # ---------------- mode-sum: k[32a+b, h], two contraction-64 matmuls per channel
    # psum += GWre_h^T @ Zre_h ; psum += GWim_h^T @ Zim_h  (im plane is negated)
    ks = None
    for g in range(4):
        kp_g = p_psk.tile([32, 32, 16], dt.float32, tag="psk", name=f"kp{g}")
        for hl in range(32):
            h = 32 * g + hl
            hg, hp = h // HG, h % HG
            T.matmul(kp_g[0:32, hl, :], GWre_t[64 * hg:64 * hg + 64, hp, :],
                     Zre_t[64 * hg:64 * hg + 64, hp, :], start=True, stop=False)
            T.matmul(kp_g[0:32, hl, :], GWim_t[64 * hg:64 * hg + 64, hp, :],
                     Zim_t[64 * hg:64 * hg + 64, hp, :], start=False, stop=True)
        # evacuate (a-major, all channels in one tile) on ACT -- DVE's queue is
        # still draining the power-chain tail here and would hold the kp psum
        # ring (and with it the whole K pipeline) hostage
        if g == 0:
            ks = p_ks.tile([32, 16, 128], dt.float16, tag="ks", name="ksall")
        A.copy(ks[0:32, :, 32 * g:32 * g + 32], kp_g[:].transpose([0, 2, 1]))
    # shuffle into one l-major tile for a contraction-128 kdft: 4 partition-subrange
    # DMAs, each moving one al row-block for all chunks and channel groups
    kcb = p_kc.tile([128, 4, 128], dt.float16, tag="kc", name="kcb")
    for al in range(4):
        # two rows ride the Pool SWDGE queue (idle between the chain tail and
        # the first A-block) so the four transfers overlap across queues
        eng = (nc.sync, GP, nc.scalar, GP)[al]
        eng.dma_start(kcb[:][32 * al:32 * al + 32, :, :],
                      ks[0:32, al:16:4, :])

    # half 4's remaining matmuls cover the kcb shuffle latency
    emit_half(4, fts=range(2, NFT))

    # ---------------- K_f via packed DFT (reuse forward stationaries j=0..3).
    # The skip connection u*D folds into the filter as K'_f = K_f + D/SCALE: a
    # rank-1 ones^T x (D/SCALE) matmul accumulated into each cos psum group.
    dt4 = p_small.tile([1, HS], dt.float32, tag="dt4")
    V.tensor_scalar_mul(dt4[:], dtile, 1.0 / SCALE)
    dt16 = p_small.tile([1, HS], dt.float16, tag="dt16")
    V.tensor_copy(dt16[:], dt4[:])
    ones = p_small.tile([1, 128], dt.float16, tag="ones")
    V.memset(ones[:], 1.0)
    kdft_ps = {}
    pks = {}
    for t_ in range(2):
        pks[t_] = p_psk.tile([128, 4, 128], dt.float32, tag="psk", name=f"kdft{t_}")
    for ft in range(NFT):
        for t_ in range(2):
            for c in range(4):
                T.matmul(pks[t_][:, ft, :], cf_tile(c, t_, ft), kcb[:, c, :],
                         start=(c == 0), stop=(c == 3 and t_ == 1))
            if t_ == 0:
                # fold the skip connection: K'_f = K_f + D/SCALE on every cos row
                T.matmul(pks[t_][:, ft, :], ones[:], dt16[:], start=False, stop=True)
            kdft_ps[(t_, ft)] = pks[t_][:, ft, :]

    zrow = p_small.tile([1, 128], dt.float32, tag="zrow")
    V.memset(zrow[:], 0.0)
    # K tiles assemble as ft-PAIR tensors [128, 2, 128] so each pointwise
    # product is one wide op with a 4D-broadcast multiplier. krD2 (the Us-side
    # multiplier of pair 0) is krA's pair 0 with partition-0 slot 0 replaced by
    # K512r + D/SCALE; pair 1's Us multiplier is krA2[1] itself.
    # the K copies run on DVE (idle here): on ACT they would sit ahead of
    # half 5's psf evacuations and stall PE's forward stream
    krA2 = [p_krep.tile([128, 2, 128], dt.float16, tag=f"krA2{p}", name=f"krA2{p}")
            for p in range(2)]
    krBC2 = [p_krep.tile([128, 2, 128], dt.float16, tag=f"krB2{p}", name=f"krB2{p}")
             for p in range(2)]
    krD2 = p_krep.tile([128, 2, 128], dt.float16, tag="krD2")
    V.tensor_copy(krD2[:, 0, :], kdft_ps[(0, 0)])
    # slot-0 partition-0 of the D-tensor holds K512r (packed sin psum row 0)
    # plus D/SCALE
    TT(krD2[0:1, 0, :], kdft_ps[(1, 0)][0:1, :], dt4[:], op.add)
    V.tensor_copy(krD2[:, 1, :], kdft_ps[(0, 1)])
    for e in range(2):
        V.tensor_copy(krA2[0][:, e, :], kdft_ps[(0, e)])
        V.tensor_copy(krBC2[0][:, e, :], kdft_ps[(1, e)])
    # pair 1's copies ride ACT (free between half 4's evacuations and B0's):
    # on DVE they would sit ahead of A(0)'s pointwise and delay the first
    # inverse block
    for e in range(2):
        A.copy(krA2[1][:, e, :], kdft_ps[(0, 2 + e)])
        A.copy(krBC2[1][:, e, :], kdft_ps[(1, 2 + e)])
    V.tensor_scalar(krBC2[0][0:1, 0, :], zrow[:], 0.0, 0.0, op.mult, op.add)

    # ---------------- main loop: software-pipelined overlap-save blocks.
    # A(blk) = combine + pointwise on DVE/Pool (runs ~1 block ahead);
    # B(blk) = inverse matmuls on PE + ACT evacuation + y DMA.

    def kb2(t):
        return t[:].unsqueeze(2).broadcast_to([128, 2, 4, 128])

    yrs = {}

    def emit_A(blk):
        yr_t, yi_t = [], []
        for p in range(2):
            # combine half spectra, both fts of the pair and cos+sin at once:
            # U_b = P_{b-1} + P_b (even pair) / P_{b-1} - P_b (odd pair)
            pcur = halves[blk][p]
            if blk == 0:
                if p == 0:
                    up = pcur
                else:
                    up = p_u.tile([128, 2, 1024], dt.float16, tag="u", name=f"u{blk}_{p}")
                    V.tensor_scalar_mul(up[:].rearrange("p a b -> p (a b)"),
                                        pcur[:].rearrange("p a b -> p (a b)"), -1.0)
            else:
                pprev = halves[blk - 1][p]
                cop = op.add if p == 0 else op.subtract
                up = p_u.tile([128, 2, 1024], dt.float16, tag="u", name=f"u{blk}_{p}")
                TT(up[:].rearrange("p a b -> p (a b)"),
                   pprev[:].rearrange("p a b -> p (a b)"),
                   pcur[:].rearrange("p a b -> p (a b)"), cop)
            uc = up[:, :, 0:512].rearrange("p a (b h) -> p a b h", b=4)
            us = up[:, :, 512:1024].rearrange("p a (b h) -> p a b h", b=4)

            # pointwise products, one wide op per (pair, term):
            # Yr = Uc*A - Us*BC ; Yi = Uc*BC + Us*D. Pool takes t2 (both
            # pairs) and t3 of pair 0 -- its 2222 vs DVE's 593 ns/op balances
            # the engines at ~7.6us per block.
            dten = krD2 if p == 0 else krA2[1]
            t1 = p_tmp.tile([128, 2, 4, 128], dt.float16, tag="t1")
            t2 = p_tmp.tile([128, 2, 4, 128], dt.float16, tag="t2")
            TT(t1[:], uc, kb2(krA2[p]), op.mult)
            # Pool's ops and the sums run at per-ft granularity: a pair-wide
            # 2222ns Pool op in the middle of the combine->product->sum chain
            # (and a sum writing the whole pair tile) holds the inverse
            # matmuls of BOTH fts hostage
            for e in range(2):
                GT(t2[:, e], us[:, e], kb2(krBC2[p])[:, e], op.mult)
            t3 = p_tmp.tile([128, 2, 4, 128], dt.float16, tag="t1")
            t4 = p_tmp.tile([128, 2, 4, 128], dt.float16, tag="t2")
            TT(t4[:], us, kb2(dten), op.mult)
            t3eng = GT if p == 0 else TT
            if t3eng is GT:
                for e in range(2):
                    GT(t3[:, e], uc[:, e], kb2(krBC2[p])[:, e], op.mult)
            else:
                TT(t3[:], uc, kb2(krBC2[p]), op.mult)
            yr = p_yf.tile([128, 2, 512], dt.float16, tag="yf")
            yi = p_yf.tile([128, 2, 512], dt.float16, tag="yf")
            for e in range(2):
                TT(yr[:, e], t1[:, e].rearrange("p b h -> p (b h)"),
                   t2[:, e].rearrange("p b h -> p (b h)"), op.subtract)
                TT(yi[:, e], t3[:, e].rearrange("p b h -> p (b h)"),
                   t4[:, e].rearrange("p b h -> p (b h)"), op.add)
            yr_t.append(yr)
            yi_t.append(yi)
        yrs[blk] = (yr_t, yi_t)

    def emit_B(blk):
        yr_t, yi_t = yrs.pop(blk)

        def mv(tt, ft):
            return tt[ft // 2][:, ft % 2, :]

        for lt in range(NLT):
            c_out = 4 * blk + lt
            if blk == NBLK - 1 and lt == NLT - 1:
                # the very last output tile: run it as two half-free accumulation
                # groups in SEPARATE psum tiles (same bank would serialize the
                # second group behind the first copy) so the first half's
                # copy + DMA overlap the second half's matmuls
                # uneven 384/128 split: the final piece's copy + transfer are
                # tiny, so the post-last-matmul critical path is mostly the
                # fixed DMA-issue + semaphore latency
                for (c0, cw) in ((0, 448), (448, 64)):
                    sl = slice(c0, c0 + cw)
                    ph = p_psi.tile([128, 512], dt.float32, tag="psi")
                    for ft in range(NFT):
                        T.matmul(ph[:, 0:cw], ai_tile(0, ft, lt), mv(yr_t, ft)[:, sl],
                                 start=(ft == 0), stop=False)
                        T.matmul(ph[:, 0:cw], ai_tile(1, ft, lt), mv(yi_t, ft)[:, sl],
                                 start=False, stop=(ft == NFT - 1))
                    yo = p_yout.tile([128, 448], dt.float16, tag="youts")
                    A.copy(yo[:, 0:cw], ph[:, 0:cw])
                    nc.sync.dma_start(
                        y_ap[c_out].rearrange("p b h -> p (b h)")[:, sl],
                        yo[:, 0:cw])
                continue
            py = p_psi.tile([128, 512], dt.float32, tag="psi")
            for ft in range(NFT):
                T.matmul(py[:], ai_tile(0, ft, lt), mv(yr_t, ft),
                         start=(ft == 0), stop=False)
                T.matmul(py[:], ai_tile(1, ft, lt), mv(yi_t, ft),
                         start=False, stop=(ft == NFT - 1))
            yo = p_yout.tile([128, 512], dt.float16, tag="yout")
            A.copy(yo[:], py[:])
            # odd lt on SP: its dge-delay is 134ns shorter, which matters for
            # the very last transfer's latency
            eng = nc.scalar if lt % 2 == 0 else nc.sync
            eng.dma_start(y_ap[c_out], yo[:].rearrange("p (b h) -> p b h", b=4))

    emit_A(0)
    emit_half(5)
    emit_B(0)
    emit_A(1)
    emit_half(6)
    emit_B(1)
    emit_A(2)
    # half 7's pm slots reuse halves 0/1, dead after A(1)/A(2)'s combines
    emit_half(7)
    emit_B(2)
    for blk in range(3, NBLK):
        emit_A(blk)
        emit_B(blk)


def _build_program():
    if _prog.built:
        return
    nc = bacc.Bacc("TRN2", target_bir_lowering=False, debug=False,
                   num_devices=NCORES)
    aps = {}
    aps["u"] = nc.dram_tensor("u", [NCH // 2, 128, 2, 2, 4, 128], dt.float8e4,
                              kind="ExternalInput").ap()
    aps["par"] = nc.dram_tensor("par", [1, 384], dt.float32, kind="ExternalInput").ap()
    aps["W2"] = nc.dram_tensor("W2", [128, 2, HG], dt.float32, kind="ExternalInput").ap()
    aps["CF"] = nc.dram_tensor("CF", [NJ, 128, 2, NFT, 128], dt.float16,
                               kind="ExternalInput").ap()
    aps["CF8"] = nc.dram_tensor("CF8", [NFT, 128, 2, NJ, 2, 128], dt.float8e4,
                                kind="ExternalInput").ap()
    aps["AI"] = nc.dram_tensor("AI", [NFT, 128, 2, NLT, 128], dt.float16,
                               kind="ExternalInput").ap()
    aps["y"] = nc.dram_tensor("y", [NCH, 128, 4, 128], dt.float16, kind="ExternalOutput").ap()
    with tile.TileContext(nc, trace_sim=False) as tc:
        with ExitStack() as ctx:
            _emit_kernel(nc, tc, ctx, aps)
    nc.compile()
    _prog.nc = nc
    _prog.CF, _prog.AI, _prog.CF8 = build_constants()
    _prog.built = True


def _splitH(arr_nh):
    # [N, HS] -> [(hg n), HG]: rows 0-63 = channels 0-63, rows 64-127 = 64-127
    a = np.ascontiguousarray(arr_nh, dtype=f32)
    return np.ascontiguousarray(np.concatenate([a[:, 0:HG], a[:, HG:2 * HG]], axis=0))


def make_in_maps(u, D, log_dt, W_re, W_im, Lambda_re, Lambda_im):
    _build_program()
    in_maps = []
    u = np.asarray(u, dtype=f32)
    for c in range(NCORES):
        h0 = c * HS
        # chunk-pair-major u: [pair, part(128), e(2), b, h], l = 256p + 128e + part;
        # split into fp8 value + fp8 residual for the DoubleRow forward
        uc = u[:, :, h0:h0 + HS].reshape(B, NCH // 2, 2, 128, HS).transpose(1, 3, 2, 0, 4)
        uc = np.ascontiguousarray(uc)
        u8 = uc.astype(f8)
        r8 = (uc - u8.astype(f32)).astype(f8)
        u8r8 = np.ascontiguousarray(np.stack([u8, r8], axis=2))
        par = np.concatenate([
            np.asarray(log_dt[h0:h0 + HS], dtype=f32).ravel(),
            np.asarray(Lambda_re, dtype=f32).ravel(),
            np.asarray(Lambda_im, dtype=f32).ravel(),
            np.asarray(D[h0:h0 + HS], dtype=f32).ravel(),
        ]).reshape(1, 384)
        w2 = np.stack([_splitH(W_re[h0:h0 + HS].T), _splitH(W_im[h0:h0 + HS].T)], axis=1)
        in_maps.append({
            "u": u8r8,
            "par": par,
            "W2": np.ascontiguousarray(w2),
            "CF": _prog.CF,
            "CF8": _prog.CF8,
            "AI": _prog.AI,
        })
    return in_maps


LAST_RESULTS = None


def kernel(u, D, Lambda_re, Lambda_im, log_dt, W_re, W_im):
    global LAST_RESULTS
    from concourse.bass_utils import run_bass_kernel_spmd
    in_maps = make_in_maps(u, D, log_dt, W_re, W_im, Lambda_re, Lambda_im)
    res = run_bass_kernel_spmd(_prog.nc, in_maps, core_ids=list(range(NCORES)))
    LAST_RESULTS = res
    outs = []
    for c in range(NCORES):
        yc = np.asarray(res.results[c]["y"])      # [NCH, 128, B, HS] fp16
        outs.append(yc.transpose(2, 0, 1, 3).reshape(B, L, HS))
    y = np.concatenate(outs, axis=2)
    return y.astype(np.float32)


# revision 89
# speedup vs baseline: 1.0071x; 1.0071x over previous
"""DSS (Diagonal State Space) layer as a Bass/Tile kernel for 8 Trainium2 NeuronCores.

Algorithm (per core, channels H sharded 8 x 128):
  1. Build the DSS-exp kernel k[l,h] = Re(sum_n W[h,n] z[h,n]^l), z = exp(dt_h * Lambda_n),
     on-device via a two-level power factorization l = 32a + b:
       GW[h,n,b] = W * z^b (b<32),  Z32[h,n,a] = z^(32a) (a<16), both by complex doubling,
     then a per-channel PE matmul contracts the 64 modes. The whole prologue runs in a
     split-H layout (partitions = (h//64, n)) so every elementwise op uses all 128
     partitions; chain work is balanced DVE/Pool so both finish together (~17us each).
  2. K_f = rfft_1024(k) via PE matmuls against host-precomputed DFT tiles.
  3. Overlap-save FFT convolution with HALF-WINDOW SHARING: each 512-sample half-window's
     1024-point DFT contribution P_m is computed once (4 chunk matmuls x 2 parts x 4 freq
     tiles) and reused by the two blocks that contain it: U_b = P_{b-1} + (-1)^f P_b.
     Frequencies are packed even/odd (tiles 0,1 = even f, tiles 2,3 = odd f) so the
     (-1)^f factor is a constant +1/-1 per tile and the combine is one fp16 add/subtract.
  4. The skip connection y += u * D is folded into the frequency-domain filter
     (K'_f = K_f + D), so it costs nothing in the main loop.

Precision plan (rel err ~3.7e-3 vs the 2e-2 gate):
  - The FORWARD DFT runs in fp8-e4m3 DoubleRow matmuls (0.5 cycles/row, 2x fp16
    rate) with a residual split: u = u8 + r8 (both e4m3, one merged transfer),
    CF = CFhi + CFlo. P = (CFhi|CFlo)^T(u8,u8) + (CFhi_c0|CFhi_c1)^T(r8_c0,r8_c1)
    per chunk pair -- 6 DoubleRow matmuls per 256-col psum region, 25% less PE
    time than fp16, with ~3e-3 spectral error instead of fp8's raw ~3e-2.
  - Everything else runs fp16 (same PE/DVE rates as bf16, 8x less quantization
    error). A global 1/4 scale is folded into CF and compensated x16 in the
    inverse AI stationaries, keeping products < fp16 max. The mode-sum operands
    (GW, Z32 planes) are fp16 too: fp16 matmuls take 1 cycle/row where float32r
    small-free matmuls took 4 (the power/trig chains stay fp32 -- only the final
    plane writes quantize).

Pointwise runs on ft-PAIR tensors (pm/up/t/yf are [128, 2, ...]): combines,
products (4D-broadcast K-pair multipliers) and sums are 1024-2048-element fused
ops, which cuts DVE/Pool op count ~2x and element work ~8%.

Host-side layouts are chunk-major and fully contiguous, so every DMA moves
>=512B lines at full bus rate. Because HWDGE issues all hardware-DGE DMAs
through ONE serial ~630ns/slot resource, transfers are merged: params ride one
[1,384] row, u8+r8 load as one tile per chunk pair, CF8 as per-ft mega-tiles
(whose hl axis strides across chunks, so the cross-chunk hi-pair stationary is
a free view), Wre/Wim as one tile, AI as per-ft pairs.

Schedule: warm-up matmuls keep PE busy from ~1.1us so the p-state ramp is paid
once. The K pipeline is front-loaded: mode-sum splits half 4 (the Z32 chain
tail ends ~37us), kp evacuations ride ACT, K copies ride DVE (each would stall
PE behind the other engine's queue otherwise), kdft right after. The block loop
is software-pipelined: A(blk) = combine+pointwise on DVE/Pool runs a block
ahead of B(blk) = inverse matmuls + ACT evacuation + y DMA; halves 5-7 slot
between the first inverse blocks as PE-side cushions while the A-stream primes.
The last output tile runs as two uneven accumulation groups in separate psum
banks so the final DMA's fixed issue+semaphore latency dominates the tail.
"""

import sys

for _p in ("/opt/trn_rl_repo", "/opt/trn_rl_repo/concourse"):
    if _p not in sys.path:
        sys.path.insert(0, _p)

import numpy as np
import ml_dtypes
from contextlib import ExitStack

import concourse.bacc as bacc
import concourse.tile as tile
import concourse.mybir as mybir

dt = mybir.dt
f32 = np.float32
f16 = np.float16
f8 = ml_dtypes.float8_e4m3
DR = mybir.MatmulPerfMode.DoubleRow

B, L, H, N = 4, 4096, 1024, 64
LK = 512
F = 1024          # FFT length (overlap-save)
HOP = 512         # block hop
NCORES = 8
HS = H // NCORES  # 128 channels per core
NBLK = L // HOP   # 8
NFT = 4           # packed frequency tiles (even: 0,1; odd: 2,3; Nyquist folded)
NJ = 4            # contraction chunks per half-window
NLT = HOP // 128  # 4 output l-tiles per block
NCH = L // 128    # 32 u chunks per core
NHALF = L // HOP  # 8 half-windows
HG = 64           # channels per split-H partition group (prologue layout)
SCALE = 4.0       # CF * 1/SCALE, AI * SCALE^2, D-injection * 1/SCALE
N_WARM0 = 43      # PE warm-up matmuls before the first real matmul
N_WARMJ = 0       # gap-filler matmuls between half-0 j-groups


# ---------------------------------------------------------------- host constants
def _freq_perm():
    perm = np.zeros((NFT, 128), dtype=np.int64)
    r = np.arange(128)
    perm[0] = 2 * r
    perm[1] = 2 * (128 + r)
    perm[2] = 2 * r + 1
    perm[3] = 2 * (128 + r) + 1
    return perm


def build_constants():
    perm = _freq_perm()
    l = np.arange(HOP, dtype=np.float64)
    CF = np.zeros((NJ, 2, NFT, 128, 128))
    for j in range(NJ):
        lj = l[128 * j:128 * j + 128][:, None]
        for ft in range(NFT):
            f = perm[ft][None, :].astype(np.float64)
            ang = 2 * np.pi * lj * f / F
            CF[j, 0, ft] = np.cos(ang)
            CF[j, 1, ft] = -np.sin(ang)
        # Nyquist row packed into the f=0 slot of the sin tile (even group, sign +1)
        CF[j, 1, 0][:, 0] = (-1.0) ** l[128 * j:128 * j + 128]
    lc = HOP + np.arange(HOP, dtype=np.float64)[None, :]   # valid circular outputs
    AI = np.zeros((2, NFT, NLT, 128, 128))
    for ft in range(NFT):
        fr = perm[ft][:, None].astype(np.float64)
        cf_ = np.where(fr == 0, 1.0, 2.0)
        Ar = cf_ * np.cos(2 * np.pi * fr * lc / F) / F
        Ai = -(2.0 / F) * np.sin(2 * np.pi * fr * lc / F)
        if ft == 0:
            Ai[0, :] = ((-1.0) ** lc[0]) / F                # Nyquist inverse row
        for lt in range(NLT):
            AI[0, ft, lt] = Ar[:, 128 * lt:128 * lt + 128]
            AI[1, ft, lt] = Ai[:, 128 * lt:128 * lt + 128]
    CF *= 1.0 / SCALE
    AI *= SCALE * SCALE
    # SBUF-tile-major merged layouts (one contiguous DMA each):
    # CF [j, l(128), t, ft, f] (fp16, kdft only) ; AI [ft, f(128), t, lt, l]
    CFd = np.ascontiguousarray(CF.transpose(0, 3, 1, 2, 4)).astype(f16)
    AId = np.ascontiguousarray(AI.transpose(1, 3, 0, 2, 4)).astype(f16)
    # fp8 hi/lo residual split of CF for the DoubleRow forward:
    # CF8 [ft, l(128), t, j, hl(2), f]
    CFhi = CF.astype(f8).astype(np.float64)
    CFlo = (CF - CFhi).astype(f8)
    CF8 = np.stack([CFhi.astype(f8), CFlo], axis=4)   # [j, t, ft, 128, hl, f]
    CF8d = np.ascontiguousarray(CF8.transpose(2, 3, 1, 0, 4, 5))
    return CFd, AId, CF8d


# Horner coefficient lists (highest degree first)
def _fact(k):
    r = 1.0
    for i in range(2, k + 1):
        r *= i
    return r


EXP10 = [1.0 / _fact(k) for k in range(7, -1, -1)]          # e^x, |x| <~ 0.9
EXP9 = [1.0 / _fact(k) for k in range(6, -1, -1)]            # e^x, |x| <~ 0.4
SIN9 = [1.0 / _fact(9), -1.0 / _fact(7), 1.0 / _fact(5), -1.0 / _fact(3), 1.0]   # odd, in u = x^2
COSC = [-1.0 / _fact(10), 1.0 / _fact(8), -1.0 / _fact(6), 1.0 / _fact(4),
        -1.0 / _fact(2), 1.0]     # cos(x) - 1 in u = x^2, b1 first


class _Prog:
    def __init__(self):
        self.nc = None
        self.built = False


_prog = _Prog()


def _emit_kernel(nc, tc, ctx, aps):
    V = nc.vector
    A = nc.scalar
    T = nc.tensor
    GP = nc.gpsimd
    u_ap = aps["u"]; y_ap = aps["y"]
    cf_ap = aps["CF"]; ai_ap = aps["AI"]
    TT = V.tensor_tensor
    GT = GP.tensor_tensor
    op = mybir.AluOpType

    # ---------------- pools
    p_cf = ctx.enter_context(tc.tile_pool(name="cf", bufs=1))
    p_ai = ctx.enter_context(tc.tile_pool(name="ai", bufs=1))
    p_uch = ctx.enter_context(tc.tile_pool(name="uch", bufs=6))
    p_pm = ctx.enter_context(tc.tile_pool(name="pm", bufs=14))
    p_u = ctx.enter_context(tc.tile_pool(name="u", bufs=6))
    p_yf = ctx.enter_context(tc.tile_pool(name="yf", bufs=10))
    p_krep = ctx.enter_context(tc.tile_pool(name="krep", bufs=1))
    p_tmp = ctx.enter_context(tc.tile_pool(name="tmp", bufs=6))
    p_yout = ctx.enter_context(tc.tile_pool(name="yout", bufs=3))
    p_kc = ctx.enter_context(tc.tile_pool(name="kc", bufs=1))
    p_gw = ctx.enter_context(tc.tile_pool(name="gw", bufs=1))
    p_z32 = ctx.enter_context(tc.tile_pool(name="z32", bufs=1))
    p_zp = ctx.enter_context(tc.tile_pool(name="zp", bufs=9))
    p_small = ctx.enter_context(tc.tile_pool(name="small", bufs=1))
    p_gwtmp = ctx.enter_context(tc.tile_pool(name="gwtmp", bufs=1))
    p_ks = ctx.enter_context(tc.tile_pool(name="ks", bufs=1))
    p_psf = ctx.enter_context(tc.tile_pool(name="psf", bufs=2, space="PSUM"))
    p_psi = ctx.enter_context(tc.tile_pool(name="psi", bufs=2, space="PSUM"))
    p_psk = ctx.enter_context(tc.tile_pool(name="psk", bufs=2, space="PSUM"))

    # ---------------- PE warm-up: keep the tensor engine "continuously busy"
    # from its first possible slot so the p-state ramp completes once.
    wmS = p_small.tile([1, 64], dt.float16, tag="wmS")
    GP.memset(wmS[:], 1.0)
    wmM = p_small.tile([1, 64], dt.float16, tag="wmM")
    GP.memset(wmM[:], 1.0)
    ps_w = p_psk.tile([64, 64], dt.float32, tag="psk", name="warm")

    def warm(n):
        for _ in range(n):
            T.matmul(ps_w[:], wmS[:], wmM[:], start=True, stop=True)

    warm(N_WARM0)

    # ---------------- startup DMAs: HWDGE serializes ALL hwdge queues at one
    # ~630ns slot per DMA, so the startup set is merged into few transfers,
    # ordered by first PE need. The Pool SWDGE queue is left empty so the
    # Lambda chain gets the Pool engine immediately.
    par = p_small.tile([1, 384], dt.float32, tag="par")
    A.dma_start(par[:], aps["par"][:])
    logdt = par[:, 0:HS]
    Lre = par[:, HS:HS + N]
    Lim_v = par[:, HS + N:HS + 2 * N]
    dtile = par[:, HS + 2 * N:HS + 2 * N + HS]

    cf_big = {}

    def load_cf(j, eng):
        tl = p_cf.tile([128, 2, 4, 128], dt.float16, tag=f"cf{j}", name=f"cfb{j}")
        eng.dma_start(tl[:], cf_ap[j])
        cf_big[j] = tl

    def cf_tile(j, t_, ft):
        return cf_big[j][:, t_, ft, :]

    cf8_big = {}

    def load_cf8(ft, eng):
        tl = p_cf.tile([128, 2, 4, 2, 128], dt.float8e4, tag=f"cf8{ft}", name=f"cf8b{ft}")
        eng.dma_start(tl[:], aps["CF8"][ft])
        cf8_big[ft] = tl

    def cf8_hl(ft, t_, j):
        # DoubleRow stationary (CFhi, CFlo) for chunk j
        return cf8_big[ft][:, t_, j, :, :]

    def cf8_hh(ft, t_, jp):
        # DoubleRow stationary (CFhi_c2p, CFhi_c2p+1): the hl axis of the
        # mega-tile strides across j, so the cross-chunk hi pair is a free view
        return cf8_big[ft][:, t_, 2 * jp:2 * jp + 2, 0, :]

    chunks = {}

    def get_chunk(c, eng=None):
        # one merged [u8 | r8] transfer per chunk pair
        p = c // 2
        if p not in chunks:
            t8 = p_uch.tile([128, 2, 2, 4, 128], dt.float8e4, tag="uch", name=f"u8p{p}")
            if eng is None:
                # SP only: a chunk DMA on ACT's SEQ delays the psf evacuations
                # that pace the forward
                eng = nc.sync
            eng.dma_start(t8[:], u_ap[p])
            chunks[p] = t8
        return chunks[p]

    load_cf8(0, nc.sync)
    get_chunk(0, nc.sync)       # chunk pair 0
    load_cf8(1, nc.scalar)
    get_chunk(2, nc.sync)       # pair 1 right behind pair 0 on SP
    load_cf8(2, GP)             # cf8 pair 2 via Pool SWDGE
    load_cf8(3, nc.sync)
    get_chunk(4, nc.sync)       # prefetch half 1's pairs
    get_chunk(6, nc.scalar)

    # ---------------- forward: one half-window spectrum P_m per 512 samples.
    # P is stored in FREQ-TILE-PAIR tiles [128, 2, 1024] (pairs (ft0,ft1) and
    # (ft2,ft3)): the combine/pointwise then run as wide fused ops and the
    # (-1)^f combine sign is constant per pair.
    halves = {}

    def emit_half(m, fts=range(NFT), pad=0):
        # cos/sin accumulation groups share one 2-bank psum tile per ft, so the
        # evacuation is a single wide ACT copy
        pair = halves.setdefault(m, {})
        for ft in fts:
            pp = p_psf.tile([128, 1024], dt.float32, tag="psf", name=f"pp{m}_{ft}")
            # the cos half [0:512] and sin half [512:1024] are different psum
            # banks, so their 256-col accumulation groups can run interleaved
            # j-major -- the matmul stream then consumes chunks in DMA-arrival
            # order instead of needing the whole half-window up front
            for h in range(2):
                def reg(t_):
                    return pp[:, 512 * t_ + 256 * h:512 * t_ + 256 * h + 256]

                for j in range(NJ):
                    u8t = get_chunk(4 * m + j)
                    u8c = u8t[:, 0, j % 2, :, :].rearrange("p b h -> p (b h)")
                    mov = (u8c[:, 256 * h:256 * h + 256].unsqueeze(1)
                           .broadcast_to([128, 2, 256]))
                    for t_ in range(2):
                        T.matmul(reg(t_), cf8_hl(ft, t_, j), mov,
                                 start=(j == 0), stop=False, perf_mode=DR)
                for jp in range(2):
                    r8t = get_chunk(4 * m + 2 * jp)
                    r8c = r8t[:, 1, :, :, :].rearrange("p c b h -> p c (b h)")
                    for t_ in range(2):
                        T.matmul(reg(t_), cf8_hh(ft, t_, jp),
                                 r8c[:, :, 256 * h:256 * h + 256],
                                 start=False, stop=(jp == 1), perf_mode=DR)
                if pad:
                    warm(pad)
            if ft // 2 not in pair:
                pair[ft // 2] = p_pm.tile([128, 2, 1024], dt.float16, tag="pm",
                                          name=f"pm{m}_{ft // 2}")
            A.copy(pair[ft // 2][:, ft % 2, :], pp[:])

    # half 0 ft0 alone first: its j-groups arrive at the startup DMA rate, so a
    # narrow consumer (2 matmuls per j + warm padding) avoids long PE stalls
    emit_half(0, fts=range(0, 1), pad=N_WARMJ)

    def horner_exp(dst, x, coefs, eng=None, tag="horner"):
        # dst = e^x = 1 + x*Q(x). On DVE each Horner step p <- (p + b)*x is one
        # fused scalar_tensor_tensor op; the Pool engine has no STT on hardware,
        # so it uses the classic two-op form there.
        tsm = eng.tensor_scalar_mul if eng is not None else V.tensor_scalar_mul
        tsa = eng.tensor_scalar_add if eng is not None else V.tensor_scalar_add
        p = dst
        tsm(p, x, float(coefs[0]))
        for c in coefs[1:-1]:
            if eng is None:
                V.scalar_tensor_tensor(p, p, float(c), x, op.add, op.mult)
            else:
                tsa(p, p, float(c))
                eng.tensor_tensor(p, p, x, op.mult)
        tsa(p, p, 1.0)

    # dt = exp(logdt) = (exp(logdt/8))^8   (DVE chain)
    x8 = p_small.tile([1, HS], dt.float32, tag="x8")
    V.tensor_scalar_mul(x8[:], logdt, 0.125)
    e8 = p_small.tile([1, HS], dt.float32, tag="e8")
    horner_exp(e8[:], x8[:], EXP10, tag="h_dt")
    dtv = p_small.tile([1, HS], dt.float32, tag="dtv")
    t_a = p_small.tile([1, HS], dt.float32, tag="sq1")
    TT(t_a[:], e8[:], e8[:], op.mult)
    t_b = p_small.tile([1, HS], dt.float32, tag="sq2")
    TT(t_b[:], t_a[:], t_a[:], op.mult)
    TT(dtv[:], t_b[:], t_b[:], op.mult)

    # -exp(Lre) = -(exp(Lre/8))^8   (Pool chain, concurrent with the dt chain)
    xl = p_small.tile([1, N], dt.float32, tag="xl")
    GP.tensor_scalar_mul(xl[:], Lre, 0.125)
    el8 = p_small.tile([1, N], dt.float32, tag="el8")
    horner_exp(el8[:], xl[:], EXP10, eng=GP, tag="h_lre")
    t_c = p_small.tile([1, N], dt.float32, tag="sq3")
    GT(t_c[:], el8[:], el8[:], op.mult)
    t_d = p_small.tile([1, N], dt.float32, tag="sq4")
    GT(t_d[:], t_c[:], t_c[:], op.mult)
    negel = p_small.tile([1, N], dt.float32, tag="negel")
    t_e = p_small.tile([1, N], dt.float32, tag="sq5")
    GT(t_e[:], t_d[:], t_d[:], op.mult)
    GP.tensor_scalar_mul(negel[:], t_e[:], -1.0)

    # half-0 ft1 first: the fp8 forward reaches here ~2us sooner than negel
    # lands, so the outer products slot in after it
    emit_half(0, fts=range(1, 2))

    # outer products in split-H layout, as early as negel allows so the trig
    # and power chains start promptly: partitions = (hg, n), hg = h//64
    ps_a = p_psk.tile([128, 512], dt.float32, tag="psk")
    ps_b = p_psk.tile([128, 512], dt.float32, tag="psk")
    for hg in range(2):
        T.matmul(ps_a[64 * hg:64 * hg + 64, 0:HG], negel[:],
                 dtv[:, HG * hg:HG * hg + HG], start=True, stop=True)
        T.matmul(ps_b[64 * hg:64 * hg + 64, 0:HG], Lim_v,
                 dtv[:, HG * hg:HG * hg + HG], start=True, stop=True)

    # remaining forward halves + inverse stationaries queue behind the outer products
    emit_half(0, fts=range(2, NFT))
    # W rides here (needed for the GW seed planes at ~12us): any earlier and it
    # delays half 1's chunk pairs in the serial HWDGE queue
    wt = p_small.tile([128, 2, HG], dt.float32, tag="wt")
    A.dma_start(wt[:], aps["W2"][:])
    wret = wt[:, 0, :]
    wimt = wt[:, 1, :]
    for m in range(1, 4):
        emit_half(m)
    # fp16 CF tiles are only contracted by the kdft (~45us in): load them
    # behind the half 1-3 chunk traffic
    load_cf(0, nc.sync)
    load_cf(1, nc.scalar)
    load_cf(2, nc.sync)
    load_cf(3, nc.scalar)
    ai_big = {}
    for ft in range(NFT):
        tl = p_ai.tile([128, 2, 4, 128], dt.float16, tag=f"ai{ft}", name=f"aib{ft}")
        nc.sync.dma_start(tl[:], ai_ap[ft])
        ai_big[ft] = tl
    # prefetch halves 4/5's chunk pairs now -- emitted any later, their HWDGE
    # slots fall behind the kcb shuffle and AI tiles and PE stalls mid-half-5
    for c in (16, 18, 20, 22):
        get_chunk(c)

    def ai_tile(t_, ft, lt):
        return ai_big[ft][:, t_, lt, :]

    # half-angle pieces on [128, HG]; everything on DVE (fused STT Horner) --
    # this short serial chain gates both power chains, and Pool's 0.42-efficiency
    # two-op Horner would stretch it 3x
    ah = p_small.tile([128, HG], dt.float32, tag="ah")
    V.tensor_scalar_mul(ah[:], ps_a[0:128, 0:HG], 0.5)   # DVE: Pool cannot read PSUM
    bh = p_small.tile([128, HG], dt.float32, tag="bh")
    V.tensor_scalar_mul(bh[:], ps_b[0:128, 0:HG], 0.5)
    ea = p_small.tile([128, HG], dt.float32, tag="ea")
    horner_exp(ea[:], ah[:], EXP9, tag="h_ea")
    # sin(bh), cos(bh) via u = bh^2
    ub = p_small.tile([128, HG], dt.float32, tag="ub")
    TT(ub[:], bh[:], bh[:], op.mult)
    sp = p_small.tile([128, HG], dt.float32, tag="sp")
    V.tensor_scalar_mul(sp[:], ub[:], float(SIN9[0]))
    for c in SIN9[1:-1]:
        V.scalar_tensor_tensor(sp[:], sp[:], float(c), ub[:], op.add, op.mult)
    V.tensor_scalar_add(sp[:], sp[:], 1.0)
    sb = p_small.tile([128, HG], dt.float32, tag="sb")
    TT(sb[:], sp[:], bh[:], op.mult)          # sin(b/2)
    cb = p_small.tile([128, HG], dt.float32, tag="cb")
    V.tensor_scalar_mul(cb[:], ub[:], float(COSC[0]))
    for c in COSC[1:-1]:
        V.scalar_tensor_tensor(cb[:], cb[:], float(c), ub[:], op.add, op.mult)
    V.tensor_scalar_add(cb[:], cb[:], 1.0)

    wre = p_small.tile([128, HG], dt.float32, tag="wre")
    TT(wre[:], ea[:], cb[:], op.mult)
    wim = p_small.tile([128, HG], dt.float32, tag="wim")
    TT(wim[:], ea[:], sb[:], op.mult)

    # GW/Z32 plane tiles are fp16 (the mode-sum matmul runs at 1 cycle/row);
    # the z power chains below stay fp32 so squaring error does not compound.
    GWre_t = p_gw.tile([128, HG, 32], dt.float16, tag="gwre")
    GWim_t = p_gw.tile([128, HG, 32], dt.float16, tag="gwim")   # stores -Im
    GWre = GWre_t[:]
    GWim = GWim_t[:]
    V.tensor_copy(GWre[:, :, 0], wret)
    V.tensor_scalar_mul(GWim[:, :, 0], wimt, -1.0)

    # complex squaring on separate re/im planes (all base-partition 0, lane-aligned)
    def csq_parts(dre, dim_, sre, sim, eng=None):
        # re' = (re+im)(re-im), im' = (2 re) im -- 4-5 ops (STT fused on DVE;
        # the Pool engine has no STT on hardware)
        tt = eng.tensor_tensor if eng is not None else TT
        sfx = "p" if eng is not None else ""
        t1 = p_small.tile([128, HG], dt.float32, tag=f"csq1{sfx}", bufs=2)
        tt(t1[:], sre, sim, op.add)
        t2 = p_small.tile([128, HG], dt.float32, tag=f"csq2{sfx}", bufs=2)
        tt(t2[:], sre, sim, op.subtract)
        if eng is None:
            V.scalar_tensor_tensor(dim_, sre, 2.0, sim, op.mult, op.mult)
        else:
            t3 = p_small.tile([128, HG], dt.float32, tag=f"csq3{sfx}", bufs=2)
            tt(t3[:], sre, sim, op.mult)
            eng.tensor_scalar_mul(dim_, t3[:], 2.0)
        tt(dre, t1[:], t2[:], op.mult)

    def new_zpair(nm):
        zr = p_zp.tile([128, HG], dt.float32, tag="zp", name=f"{nm}r")
        zi = p_zp.tile([128, HG], dt.float32, tag="zp", name=f"{nm}i")
        return zr, zi

    def cdouble_seg(pre, pim, zr, zi, s0, d0, w, conj_stored, prim=None, sec=None):
        # planes [.., d0:d0+w] = planes[.., s0:s0+w] * (zr + i zi);
        # when conj_stored, the im plane holds the negated imaginary part.
        ptt = prim.tensor_tensor if prim is not None else TT
        stt = sec.tensor_tensor if sec is not None else GT
        tg = "zt2" if prim is not None else "gt2"
        zre = zr[:].unsqueeze(2).broadcast_to([128, HG, w])
        zim = zi[:].unsqueeze(2).broadcast_to([128, HG, w])
        t2 = p_gwtmp.tile([128, HG, 8], dt.float32, tag=tg, bufs=2)
        t4 = p_gwtmp.tile([128, HG, 8], dt.float32, tag=tg, bufs=2)
        ptt(pre[:, :, d0:d0 + w], pre[:, :, s0:s0 + w], zre, op.mult)
        stt(t2[:, :, 0:w], pim[:, :, s0:s0 + w], zim, op.mult)
        ptt(pim[:, :, d0:d0 + w], pim[:, :, s0:s0 + w], zre, op.mult)
        stt(t4[:, :, 0:w], pre[:, :, s0:s0 + w], zim, op.mult)
        ptt(pre[:, :, d0:d0 + w], pre[:, :, d0:d0 + w], t2[:, :, 0:w],
            op.add if conj_stored else op.subtract)
        ptt(pim[:, :, d0:d0 + w], pim[:, :, d0:d0 + w], t4[:, :, 0:w],
            op.subtract if conj_stored else op.add)

    def cdouble(pre, pim, zr, zi, w, conj_stored, prim=None, sec=None):
        cdouble_seg(pre, pim, zr, zi, 0, w, w, conj_stored, prim=prim, sec=sec)

    # ---------------- Z32 planes [(hg n), HG, 16] natural complex z^(32a)
    Zre_t = p_z32.tile([128, HG, 16], dt.float16, tag="z32re")
    Zim_t = p_z32.tile([128, HG, 16], dt.float16, tag="z32im")
    Zre = Zre_t[:]
    Zim = Zim_t[:]
    # a=0 plane is the complex constant 1+0i
    V.tensor_scalar(Zre[:, :, 0], wre[:], 0.0, 1.0, op.mult, op.add)
    V.tensor_scalar(Zim[:, :, 0], wre[:], 0.0, 0.0, op.mult, op.add)

    # interleaved power chain + doubling. Engine split tuned so DVE and Pool
    # chain streams both land ~17us: GW doubling DVE-primary/Pool-secondary,
    # Z32 levels w=1,2 Pool-primary (cheap there), w=4,8 DVE-primary (Pool's
    # 0.42-efficiency makes wide levels 3.4x dearer), za squarings on Pool.
    zp = []
    z0 = new_zpair("z0")
    csq_parts(z0[0][:], z0[1][:], wre[:], wim[:])
    zp.append(z0)
    cdouble(GWre, GWim, zp[0][0], zp[0][1], 1, conj_stored=True)
    for j in range(1, 5):                     # z^2, z^4, z^8, z^16
        zj = new_zpair(f"z{1 << j}")
        csq_parts(zj[0][:], zj[1][:], zp[-1][0][:], zp[-1][1][:])
        zp.append(zj)
        if j < 4:
            cdouble(GWre, GWim, zp[j][0], zp[j][1], 1 << j, conj_stored=True)
    # the za squarings are a SERIAL latency chain (za1 <- za0 <- ...), so they
    # run on DVE (508 vs Pool's 1110 ns per level); the wide Z32 doubling
    # levels are Pool-primary so Pool and DVE finish their chain streams
    # together (~30us)
    za = []
    z32t = new_zpair("z32")
    csq_parts(z32t[0][:], z32t[1][:], zp[4][0][:], zp[4][1][:])
    za.append(z32t)                           # z^32
    cdouble_seg(GWre, GWim, zp[4][0], zp[4][1], 0, 16, 8, conj_stored=True)
    cdouble_seg(GWre, GWim, zp[4][0], zp[4][1], 8, 24, 8, conj_stored=True)
    cdouble(Zre, Zim, za[0][0], za[0][1], 1, conj_stored=False, prim=GP, sec=V)
    for j in range(1, 4):                     # z^64, z^128, z^256
        zj = new_zpair(f"za{j}")
        csq_parts(zj[0][:], zj[1][:], za[-1][0][:], za[-1][1][:])
        za.append(zj)
        if j < 3:
            cdouble(Zre, Zim, za[j][0], za[j][1], 1 << j, conj_stored=False, prim=GP, sec=V)
        else:
            # the widest (and last) Z32 level runs DVE-primary: it is the
            # chain's tail and Pool would stretch it 3.6x
            cdouble(Zre, Zim, za[j][0], za[j][1], 1 << j, conj_stored=False, prim=V, sec=GP)

    # half 4's first freq pair keeps PE fed while the Z32 chain tail finishes;
    # the mode-sum slots in after it and the kcb shuffle DMAs then overlap
    # half 4's remaining matmuls
    emit_half(4, fts=range(0, 2))

    # ---------------- mode-sum: k[32a+b, h], two contraction-64 matmuls per channel
    # psum += GWre_h^T @ Zre_h ; psum += GWim_h^T @ Zim_h  (im plane is negated)
    ks = None
    for g in range(4):
        kp_g = p_psk.tile([32, 32, 16], dt.float32, tag="psk", name=f"kp{g}")
        for hl in range(32):
            h = 32 * g + hl
            hg, hp = h // HG, h % HG
            T.matmul(kp_g[0:32, hl, :], GWre_t[64 * hg:64 * hg + 64, hp, :],
                     Zre_t[64 * hg:64 * hg + 64, hp, :], start=True, stop=False)
            T.matmul(kp_g[0:32, hl, :], GWim_t[64 * hg:64 * hg + 64, hp, :],
                     Zim_t[64 * hg:64 * hg + 64, hp, :], start=False, stop=True)
        # evacuate (a-major, all channels in one tile) on ACT -- DVE's queue is
        # still draining the power-chain tail here and would hold the kp psum
        # ring (and with it the whole K pipeline) hostage
        if g == 0:
            ks = p_ks.tile([32, 16, 128], dt.float16, tag="ks", name="ksall")
        A.copy(ks[0:32, :, 32 * g:32 * g + 32], kp_g[:].transpose([0, 2, 1]))
    # shuffle into one l-major tile for a contraction-128 kdft: 4 partition-subrange
    # DMAs, each moving one al row-block for all chunks and channel groups
    kcb = p_kc.tile([128, 4, 128], dt.float16, tag="kc", name="kcb")
    for al in range(4):
        # two rows ride the Pool SWDGE queue (idle between the chain tail and
        # the first A-block) so the four transfers overlap across queues
        eng = (nc.sync, GP, nc.scalar, GP)[al]
        eng.dma_start(kcb[:][32 * al:32 * al + 32, :, :],
                      ks[0:32, al:16:4, :])

    # half 4's remaining matmuls cover the kcb shuffle latency
    emit_half(4, fts=range(2, NFT))

    # ---------------- K_f via packed DFT (reuse forward stationaries j=0..3).
    # The skip connection u*D folds into the filter as K'_f = K_f + D/SCALE: a
    # rank-1 ones^T x (D/SCALE) matmul accumulated into each cos psum group.
    dt4 = p_small.tile([1, HS], dt.float32, tag="dt4")
    V.tensor_scalar_mul(dt4[:], dtile, 1.0 / SCALE)
    dt16 = p_small.tile([1, HS], dt.float16, tag="dt16")
    V.tensor_copy(dt16[:], dt4[:])
    ones = p_small.tile([1, 128], dt.float16, tag="ones")
    V.memset(ones[:], 1.0)
    kdft_ps = {}
    pks = {}
    for t_ in range(2):
        pks[t_] = p_psk.tile([128, 4, 128], dt.float32, tag="psk", name=f"kdft{t_}")
    for ft in range(NFT):
        for t_ in range(2):
            for c in range(4):
                T.matmul(pks[t_][:, ft, :], cf_tile(c, t_, ft), kcb[:, c, :],
                         start=(c == 0), stop=(c == 3 and t_ == 1))
            if t_ == 0:
                # fold the skip connection: K'_f = K_f + D/SCALE on every cos row
                T.matmul(pks[t_][:, ft, :], ones[:], dt16[:], start=False, stop=True)
            kdft_ps[(t_, ft)] = pks[t_][:, ft, :]

    zrow = p_small.tile([1, 128], dt.float32, tag="zrow")
    V.memset(zrow[:], 0.0)
    # K tiles assemble as ft-PAIR tensors [128, 2, 128] so each pointwise
    # product is one wide op with a 4D-broadcast multiplier. krD2 (the Us-side
    # multiplier of pair 0) is krA's pair 0 with partition-0 slot 0 replaced by
    # K512r + D/SCALE; pair 1's Us multiplier is krA2[1] itself.
    # the K copies run on DVE (idle here): on ACT they would sit ahead of
    # half 5's psf evacuations and stall PE's forward stream
    krA2 = [p_krep.tile([128, 2, 128], dt.float16, tag=f"krA2{p}", name=f"krA2{p}")
            for p in range(2)]
    krBC2 = [p_krep.tile([128, 2, 128], dt.float16, tag=f"krB2{p}", name=f"krB2{p}")
             for p in range(2)]
    krD2 = p_krep.tile([128, 2, 128], dt.float16, tag="krD2")
    V.tensor_copy(krD2[:, 0, :], kdft_ps[(0, 0)])
    # slot-0 partition-0 of the D-tensor holds K512r (packed sin psum row 0)
    # plus D/SCALE
    TT(krD2[0:1, 0, :], kdft_ps[(1, 0)][0:1, :], dt4[:], op.add)
    V.tensor_copy(krD2[:, 1, :], kdft_ps[(0, 1)])
    for e in range(2):
        V.tensor_copy(krA2[0][:, e, :], kdft_ps[(0, e)])
        V.tensor_copy(krBC2[0][:, e, :], kdft_ps[(1, e)])
    # pair 1's copies ride ACT (free between half 4's evacuations and B0's):
    # on DVE they would sit ahead of A(0)'s pointwise and delay the first
    # inverse block
    for e in range(2):
        A.copy(krA2[1][:, e, :], kdft_ps[(0, 2 + e)])
        A.copy(krBC2[1][:, e, :], kdft_ps[(1, 2 + e)])
    V.tensor_scalar(krBC2[0][0:1, 0, :], zrow[:], 0.0, 0.0, op.mult, op.add)

    # ---------------- main loop: software-pipelined overlap-save blocks.
    # A(blk) = combine + pointwise on DVE/Pool (runs ~1 block ahead);
    # B(blk) = inverse matmuls on PE + ACT evacuation + y DMA.

    def kb2(t):
        return t[:].unsqueeze(2).broadcast_to([128, 2, 4, 128])

    yrs = {}

    def emit_A(blk):
        yr_t, yi_t = [], []
        for p in range(2):
            # combine half spectra, both fts of the pair and cos+sin at once:
            # U_b = P_{b-1} + P_b (even pair) / P_{b-1} - P_b (odd pair)
            pcur = halves[blk][p]
            if blk == 0:
                if p == 0:
                    up = pcur
                else:
                    up = p_u.tile([128, 2, 1024], dt.float16, tag="u", name=f"u{blk}_{p}")
                    V.tensor_scalar_mul(up[:].rearrange("p a b -> p (a b)"),
                                        pcur[:].rearrange("p a b -> p (a b)"), -1.0)
            else:
                pprev = halves[blk - 1][p]
                cop = op.add if p == 0 else op.subtract
                up = p_u.tile([128, 2, 1024], dt.float16, tag="u", name=f"u{blk}_{p}")
                TT(up[:].rearrange("p a b -> p (a b)"),
                   pprev[:].rearrange("p a b -> p (a b)"),
                   pcur[:].rearrange("p a b -> p (a b)"), cop)
            uc = up[:, :, 0:512].rearrange("p a (b h) -> p a b h", b=4)
            us = up[:, :, 512:1024].rearrange("p a (b h) -> p a b h", b=4)

            # pointwise products, one wide op per (pair, term):
            # Yr = Uc*A - Us*BC ; Yi = Uc*BC + Us*D. Pool takes t2 (both
            # pairs) and t3 of pair 0 -- its 2222 vs DVE's 593 ns/op balances
            # the engines at ~7.6us per block.
            dten = krD2 if p == 0 else krA2[1]
            t1 = p_tmp.tile([128, 2, 4, 128], dt.float16, tag="t1")
            t2 = p_tmp.tile([128, 2, 4, 128], dt.float16, tag="t2")
            TT(t1[:], uc, kb2(krA2[p]), op.mult)
            # Pool's ops and the sums run at per-ft granularity: a pair-wide
            # 2222ns Pool op in the middle of the combine->product->sum chain
            # (and a sum writing the whole pair tile) holds the inverse
            # matmuls of BOTH fts hostage
            for e in range(2):
                GT(t2[:, e], us[:, e], kb2(krBC2[p])[:, e], op.mult)
            t3 = p_tmp.tile([128, 2, 4, 128], dt.float16, tag="t1")
            t4 = p_tmp.tile([128, 2, 4, 128], dt.float16, tag="t2")
            TT(t4[:], us, kb2(dten), op.mult)
            t3eng = GT if p == 0 else TT
            if t3eng is GT:
                for e in range(2):
                    GT(t3[:, e], uc[:, e], kb2(krBC2[p])[:, e], op.mult)
            else:
                TT(t3[:], uc, kb2(krBC2[p]), op.mult)
            yr = p_yf.tile([128, 2, 512], dt.float16, tag="yf")
            yi = p_yf.tile([128, 2, 512], dt.float16, tag="yf")
            for e in range(2):
                TT(yr[:, e], t1[:, e].rearrange("p b h -> p (b h)"),
                   t2[:, e].rearrange("p b h -> p (b h)"), op.subtract)
                TT(yi[:, e], t3[:, e].rearrange("p b h -> p (b h)"),
                   t4[:, e].rearrange("p b h -> p (b h)"), op.add)
            yr_t.append(yr)
            yi_t.append(yi)
        yrs[blk] = (yr_t, yi_t)

    def emit_B(blk):
        yr_t, yi_t = yrs.pop(blk)

        def mv(tt, ft):
            return tt[ft // 2][:, ft % 2, :]

        for lt in range(NLT):
            c_out = 4 * blk + lt
            if blk == NBLK - 1 and lt == NLT - 1:
                # the very last output tile: run it as two half-free accumulation
                # groups in SEPARATE psum tiles (same bank would serialize the
                # second group behind the first copy) so the first half's
                # copy + DMA overlap the second half's matmuls
                # uneven 384/128 split: the final piece's copy + transfer are
                # tiny, so the post-last-matmul critical path is mostly the
                # fixed DMA-issue + semaphore latency
                for (c0, cw) in ((0, 448), (448, 64)):
                    sl = slice(c0, c0 + cw)
                    ph = p_psi.tile([128, 512], dt.float32, tag="psi")
                    for ft in range(NFT):
                        T.matmul(ph[:, 0:cw], ai_tile(0, ft, lt), mv(yr_t, ft)[:, sl],
                                 start=(ft == 0), stop=False)
                        T.matmul(ph[:, 0:cw], ai_tile(1, ft, lt), mv(yi_t, ft)[:, sl],
                                 start=False, stop=(ft == NFT - 1))
                    yo = p_yout.tile([128, 448], dt.float16, tag="youts")
                    A.copy(yo[:, 0:cw], ph[:, 0:cw])
                    nc.sync.dma_start(
                        y_ap[c_out].rearrange("p b h -> p (b h)")[:, sl],
                        yo[:, 0:cw])
                continue
            py = p_psi.tile([128, 512], dt.float32, tag="psi")
            for ft in range(NFT):
                T.matmul(py[:], ai_tile(0, ft, lt), mv(yr_t, ft),
                         start=(ft == 0), stop=False)
                T.matmul(py[:], ai_tile(1, ft, lt), mv(yi_t, ft),
                         start=False, stop=(ft == NFT - 1))
            yo = p_yout.tile([128, 512], dt.float16, tag="yout")
            A.copy(yo[:], py[:])
            # odd lt on SP: its dge-delay is 134ns shorter, which matters for
            # the very last transfer's latency
            eng = nc.scalar if lt % 2 == 0 else nc.sync
            eng.dma_start(y_ap[c_out], yo[:].rearrange("p (b h) -> p b h", b=4))

    emit_A(0)
    emit_half(5)
    emit_B(0)
    emit_A(1)
    emit_half(6)
    emit_B(1)
    emit_A(2)
    # half 7's pm slots reuse halves 0/1, dead after A(1)/A(2)'s combines
    emit_half(7)
    emit_B(2)
    for blk in range(3, NBLK):
        emit_A(blk)
        emit_B(blk)


def _build_program():
    if _prog.built:
        return
    nc = bacc.Bacc("TRN2", target_bir_lowering=False, debug=False,
                   num_devices=NCORES)
    aps = {}
    aps["u"] = nc.dram_tensor("u", [NCH // 2, 128, 2, 2, 4, 128], dt.float8e4,
                              kind="ExternalInput").ap()
    aps["par"] = nc.dram_tensor("par", [1, 384], dt.float32, kind="ExternalInput").ap()
    aps["W2"] = nc.dram_tensor("W2", [128, 2, HG], dt.float32, kind="ExternalInput").ap()
    aps["CF"] = nc.dram_tensor("CF", [NJ, 128, 2, NFT, 128], dt.float16,
                               kind="ExternalInput").ap()
    aps["CF8"] = nc.dram_tensor("CF8", [NFT, 128, 2, NJ, 2, 128], dt.float8e4,
                                kind="ExternalInput").ap()
    aps["AI"] = nc.dram_tensor("AI", [NFT, 128, 2, NLT, 128], dt.float16,
                               kind="ExternalInput").ap()
    aps["y"] = nc.dram_tensor("y", [NCH, 128, 4, 128], dt.float16, kind="ExternalOutput").ap()
    with tile.TileContext(nc, trace_sim=False) as tc:
        with ExitStack() as ctx:
            _emit_kernel(nc, tc, ctx, aps)
    nc.compile()
    _prog.nc = nc
    _prog.CF, _prog.AI, _prog.CF8 = build_constants()
    _prog.built = True


def _splitH(arr_nh):
    # [N, HS] -> [(hg n), HG]: rows 0-63 = channels 0-63, rows 64-127 = 64-127
    a = np.ascontiguousarray(arr_nh, dtype=f32)
    return np.ascontiguousarray(np.concatenate([a[:, 0:HG], a[:, HG:2 * HG]], axis=0))


def make_in_maps(u, D, log_dt, W_re, W_im, Lambda_re, Lambda_im):
    _build_program()
    in_maps = []
    u = np.asarray(u, dtype=f32)
    for c in range(NCORES):
        h0 = c * HS
        # chunk-pair-major u: [pair, part(128), e(2), b, h], l = 256p + 128e + part;
        # split into fp8 value + fp8 residual for the DoubleRow forward
        uc = u[:, :, h0:h0 + HS].reshape(B, NCH // 2, 2, 128, HS).transpose(1, 3, 2, 0, 4)
        uc = np.ascontiguousarray(uc)
        u8 = uc.astype(f8)
        r8 = (uc - u8.astype(f32)).astype(f8)
        u8r8 = np.ascontiguousarray(np.stack([u8, r8], axis=2))
        par = np.concatenate([
            np.asarray(log_dt[h0:h0 + HS], dtype=f32).ravel(),
            np.asarray(Lambda_re, dtype=f32).ravel(),
            np.asarray(Lambda_im, dtype=f32).ravel(),
            np.asarray(D[h0:h0 + HS], dtype=f32).ravel(),
        ]).reshape(1, 384)
        w2 = np.stack([_splitH(W_re[h0:h0 + HS].T), _splitH(W_im[h0:h0 + HS].T)], axis=1)
        in_maps.append({
            "u": u8r8,
            "par": par,
            "W2": np.ascontiguousarray(w2),
            "CF": _prog.CF,
            "CF8": _prog.CF8,
            "AI": _prog.AI,
        })
    return in_maps


LAST_RESULTS = None


def kernel(u, D, Lambda_re, Lambda_im, log_dt, W_re, W_im):
    global LAST_RESULTS
    from concourse.bass_utils import run_bass_kernel_spmd
    in_maps = make_in_maps(u, D, log_dt, W_re, W_im, Lambda_re, Lambda_im)
    res = run_bass_kernel_spmd(_prog.nc, in_maps, core_ids=list(range(NCORES)))
    LAST_RESULTS = res
    outs = []
    for c in range(NCORES):
        yc = np.asarray(res.results[c]["y"])      # [NCH, 128, B, HS] fp16
        outs.append(yc.transpose(2, 0, 1, 3).reshape(B, L, HS))
    y = np.concatenate(outs, axis=2)
    return y.astype(np.float32)
